# revision 16
# baseline (speedup 1.0000x reference)
"""Trainium2 Bass kernel: per-superpixel mean of CNN features + linear head.

reference computes:
    sums[s, f]  = segment_sum(features, superpixel)      # 1024 segments
    out[s, c]   = (sums[s] / max(count_s, 1)) @ w_node.T # [1024, 21]

Key algebraic restructure: project each pixel's 256-dim feature to the
22-dim augmented class space FIRST (21 classes + a ones-column that
yields the segment counts), then segment-sum the projections:
    out[s, c] = segsum(feats @ w_aug.T)[s, c] / segsum(ones)[s]
This turns the segment reduction into a [pix,22].T @ onehot[pix,1024]
matmul per 128-pixel tile, accumulated in PSUM across all tiles.

Sharding: the 512*512 = 262144 pixels are split evenly across 8 cores
(segment-sum is permutation-invariant over pixels). Each core emits a
[rows, 1024] partial (class sums + counts); the host adds the partials,
divides by counts and transposes.
"""

import numpy as np

import concourse.mybir as mybir
import concourse.tile as tile
from concourse import bacc
from concourse.bass_utils import run_bass_kernel_spmd

N_CORES = 8
P = 128
F = 256                      # feature dim
NUM_SP = 1024                # superpixel labels
C = 21                       # classes
CP = 22                      # classes padded even (fp32r needs even moving dim)
NPIX = 512 * 512
PIX_PER_CORE = NPIX // N_CORES       # 32768
CHUNK_PIX = 2048                     # pixels per DMA chunk (2 MiB)
N_CHUNKS = PIX_PER_CORE // CHUNK_PIX  # 16
TILES_PER_CHUNK = CHUNK_PIX // P      # 16
N_TILES = PIX_PER_CORE // P           # 256
FREE_PER_CHUNK = CHUNK_PIX * F // P   # 4096

F32 = mybir.dt.float32
F32R = mybir.dt.float32r  # fp32 layout, full-rate PE path
F16 = mybir.dt.float16
I16 = mybir.dt.int16

# segment-sum matmuls rotate over PE column-tiling groups so consecutive
# tiles' matmuls overlap in disjoint 32-column strips of the array
N_GROUPS = 4


def _build_nc():
    nc = bacc.Bacc("TRN2", target_bir_lowering=False)

    feats = nc.dram_tensor(
        "feats", [N_CHUNKS, P, FREE_PER_CHUNK], F32R, kind="ExternalInput"
    )
    labels = nc.dram_tensor("labels", [P, N_TILES], F32, kind="ExternalInput")
    iota = nc.dram_tensor("iota", [P, NUM_SP], I16, kind="ExternalInput")
    w_aug = nc.dram_tensor("w_aug", [2 * P, CP], F32R, kind="ExternalInput")
    ident_d = nc.dram_tensor("ident", [P, P], F32R, kind="ExternalInput")
    out = nc.dram_tensor("out", [P, NUM_SP], F32, kind="ExternalOutput")

    with tile.TileContext(nc) as tc:
        with (
            tc.tile_pool(name="const", bufs=1) as const_pool,
            tc.tile_pool(name="chunk", bufs=2) as chunk_pool,
            tc.tile_pool(name="work", bufs=3) as work_pool,
            tc.tile_pool(name="psum", bufs=2, space="PSUM") as psum_pool,
            tc.tile_pool(name="accp", bufs=1, space="PSUM") as acc_pool,
        ):
            ident = const_pool.tile([P, P], F32R)
            nc.sync.dma_start(out=ident[:], in_=ident_d[:])
            iota_sb = const_pool.tile([P, NUM_SP], I16)
            nc.sync.dma_start(out=iota_sb[:], in_=iota[:])
            labels_sb = const_pool.tile([P, N_TILES], F32)
            nc.sync.dma_start(out=labels_sb[:], in_=labels[:])
            w_sb = const_pool.tile([P, 2 * CP], F32R)
            nc.sync.dma_start(out=w_sb[:, 0:CP], in_=w_aug[0:P, :])
            nc.sync.dma_start(out=w_sb[:, CP : 2 * CP], in_=w_aug[P : 2 * P, :])

            # persistent accumulator: group j accumulates into rows
            # [32j, 32j+CAUG) across its subset of pixel tiles
            acc = acc_pool.tile([P, NUM_SP], F32)

            for c in range(N_CHUNKS):
                feats_sb = chunk_pool.tile([P, FREE_PER_CHUNK], F32R, tag="feats")
                nc.sync.dma_start(out=feats_sb[:], in_=feats[c])
                for t in range(TILES_PER_CHUNK):
                    tg = c * TILES_PER_CHUNK + t
                    fcol = t * F

                    # transpose the [128 pix, 256 f] tile -> [256 f, 128 pix]
                    ft_ps = psum_pool.tile([P, F], F32R, tag="ftps")
                    nc.tensor.transpose(
                        out=ft_ps[:, 0:P],
                        in_=feats_sb[:, fcol : fcol + P],
                        identity=ident[:],
                    )
                    nc.tensor.transpose(
                        out=ft_ps[:, P:F],
                        in_=feats_sb[:, fcol + P : fcol + F],
                        identity=ident[:],
                    )
                    ft_sb = work_pool.tile([P, F], F32R, tag="ftsb")
                    nc.scalar.activation(
                        out=ft_sb[:],
                        in_=ft_ps[:],
                        func=mybir.ActivationFunctionType.Copy,
                    )

                    # proj[pix, 22] = feats @ w_aug.T  (contract over features)
                    proj_ps = psum_pool.tile([P, CP], F32, tag="projps")
                    nc.tensor.matmul(
                        out=proj_ps[:],
                        lhsT=ft_sb[:, 0:P],
                        rhs=w_sb[:, 0:CP],
                        start=True,
                        stop=False,
                    )
                    nc.tensor.matmul(
                        out=proj_ps[:],
                        lhsT=ft_sb[:, P:F],
                        rhs=w_sb[:, CP : 2 * CP],
                        start=False,
                        stop=True,
                    )
                    # fp16 proj for the segment-sum matmul (values are O(1),
                    # fp16 keeps ~5e-4 relative accuracy; PSUM accumulates fp32)
                    pq_sb = work_pool.tile([P, CP], F16, tag="pqsb")
                    nc.scalar.activation(
                        out=pq_sb[:],
                        in_=proj_ps[:],
                        func=mybir.ActivationFunctionType.Copy,
                    )

                    # onehot[p, s] = (iota[p, s] == label[p]); int16 input
                    # enables the DVE 4x mode, fp16 output feeds the PE
                    onehot = work_pool.tile([P, NUM_SP], F16, tag="onehot")
                    nc.vector.tensor_scalar(
                        onehot[:],
                        iota_sb[:],
                        labels_sb[:, tg : tg + 1],
                        None,
                        mybir.AluOpType.is_equal,
                    )

                    # acc[32g + c, s] += pq[pix, c] * onehot[pix, s]
                    g = tg % N_GROUPS
                    row = 32 * g
                    first = tg < N_GROUPS
                    last = tg >= N_TILES - N_GROUPS
                    for half in range(2):
                        nc.tensor.matmul(
                            out=acc[row : row + CP, 512 * half : 512 * (half + 1)],
                            lhsT=pq_sb[:],
                            rhs=onehot[:, 512 * half : 512 * (half + 1)],
                            start=first,
                            stop=last,
                            tile_position=(0, row),
                            skip_group_check=True,
                        )

            out_sb = chunk_pool.tile([P, NUM_SP], F32, tag="outsb")
            nc.scalar.activation(
                out=out_sb[:], in_=acc[:], func=mybir.ActivationFunctionType.Copy
            )
            nc.sync.dma_start(out=out[:], in_=out_sb[:])

    nc.compile()
    return nc


def _install_ntff_hook():
    """Register the axon NTFF profiling hook when the image's antenv
    lacks axon_hooks (mirrors trn_agent_boot._ntff_profile_via_ctypes)."""
    import contextlib
    import ctypes
    import sys
    import types

    if "antenv.axon_hooks" in sys.modules:
        return
    lib = ctypes.CDLL("/opt/axon/libaxon_pjrt.so")
    if not hasattr(lib, "axon_start_nrt_profile"):
        return
    lib.axon_start_nrt_profile.argtypes = [
        ctypes.POINTER(ctypes.c_int64),
        ctypes.c_size_t,
    ]
    lib.axon_start_nrt_profile.restype = ctypes.c_int64
    lib.axon_stop_nrt_profile.argtypes = [ctypes.c_char_p]
    lib.axon_stop_nrt_profile.restype = ctypes.c_int64

    @contextlib.contextmanager
    def _hook(output_dir, device_ids):
        import jax

        jax.devices()
        if device_ids:
            ids = (ctypes.c_int64 * len(device_ids))(*device_ids)
            rc = lib.axon_start_nrt_profile(ids, len(device_ids))
        else:
            rc = lib.axon_start_nrt_profile(None, 0)
        if rc != 0:
            raise RuntimeError(f"axon_start_nrt_profile rc={rc}")
        try:
            yield
        finally:
            n = lib.axon_stop_nrt_profile(str(output_dir).encode())
            print(f"profile: {n} file(s) written to {output_dir}", file=sys.stderr)

    mod = types.ModuleType("antenv.axon_hooks")
    mod.get_axon_ntff_profile_hook = lambda: _hook
    mod.set_axon_ntff_profile_hook = lambda h: None
    sys.modules["antenv.axon_hooks"] = mod


_NC_CACHE = None


def _get_nc():
    global _NC_CACHE
    if _NC_CACHE is None:
        _NC_CACHE = _build_nc()
    return _NC_CACHE


def kernel(features, superpixel, w_node):
    features = np.ascontiguousarray(np.asarray(features, dtype=np.float32))
    superpixel = np.asarray(superpixel)
    w_node = np.asarray(w_node, dtype=np.float32)

    feats_flat = features.reshape(NPIX, F)
    sp_flat = superpixel.reshape(NPIX)

    # w_aug[f, c] layout: two stacked [128, 22] blocks of
    # [w_node.T | ones] so rhs block b is w_aug[128b:128b+128, :]
    w_aug = np.zeros((F, CP), dtype=np.float32)
    w_aug[:, :C] = w_node.T
    ident = np.eye(P, dtype=np.float32)
    iota = np.broadcast_to(
        np.arange(NUM_SP, dtype=np.int16)[None, :], (P, NUM_SP)
    ).copy()

    in_maps = []
    for core in range(N_CORES):
        lo = core * PIX_PER_CORE
        fc = feats_flat[lo : lo + PIX_PER_CORE]
        spc = sp_flat[lo : lo + PIX_PER_CORE]
        # pixel index within core = 2048*chunk + 16*partition + tile_in_chunk
        lab = (
            spc.reshape(N_CHUNKS, P, TILES_PER_CHUNK)
            .transpose(1, 0, 2)
            .reshape(P, N_TILES)
            .astype(np.float32)
        )
        in_maps.append(
            {
                "feats": fc.reshape(N_CHUNKS, P, FREE_PER_CHUNK),
                "labels": np.ascontiguousarray(lab),
                "iota": iota,
                "w_aug": w_aug,
                "ident": ident,
            }
        )

    import os

    trace = bool(int(os.environ.get("KERNEL_TRACE", "0")))
    kwargs = {}
    if trace:
        _install_ntff_hook()
        import concourse.bass_utils as _bu

        _bu.upload_artifacts = lambda tmpdir: tmpdir
        kwargs["tmpdir"] = os.environ.get("KERNEL_TRACE_DIR") or None
    res = run_bass_kernel_spmd(
        _get_nc(), in_maps, core_ids=list(range(N_CORES)), trace=trace, **kwargs
    )
    if trace:
        print(f"HW exec time: {res.exec_time_ns} ns")
        print(f"profile_json: {res.profile_json}")

    total = np.zeros((C, NUM_SP), dtype=np.float64)
    for r in res.results:
        o = np.asarray(r["out"], dtype=np.float64)
        for g in range(N_GROUPS):
            total += o[32 * g : 32 * g + C]
    counts = np.bincount(sp_flat.astype(np.int64), minlength=NUM_SP).astype(np.float64)
    node_potentials = total / np.clip(counts, 1.0, None)
    return np.ascontiguousarray(node_potentials.T).astype(np.float32)
